# revision 7
# baseline (speedup 1.0000x reference)
"""BitLinear forward on 8 Trainium2 NeuronCores (self-contained).

Reference computation (see problem):
    input_norm = layernorm(x)                     # per-token, eps=1e-5, no affine
    max_val    = absmax(input_norm)               # GLOBAL over all of x
    q          = int8-wrap(round(input_norm * 128/max_val))
    wq         = sign(W - mean(W))                # global mean
    out        = (q @ wq.T) * (max_val/128 * mean|W|) + bias

Distribution (8 cores, SPMD single NEFF):
  - Tensor-parallel: W/bias sharded along out_features (1024 rows/core);
    every core computes q for all 8192 tokens and its 1024 output columns.
  - Phase-1 stats (per-row mean/rstd + row absmax, W partial sums) are
    row-sharded (1/8 of tokens per core) and exchanged with ONE AllGather
    of 8.2 KB per core; global absmax / sum(W) / sum|W| ride along as
    extra rows and are reduced locally after the gather.
  - Matmul runs in bf16 (q in [-128,127] and wq in {-1,1} are exact in
    bf16; PE accumulates fp32 -> bit-exact integer arithmetic).

Numerics notes:
  - round() is implemented with the fp32 magic-constant trick
    (v + 1.5*2^23 - 1.5*2^23 == RNE round for |v| <= 2^22).
  - the int8 two's-complement wrap (128 -> -128) is q = round(v) - 256*(v>=127.5).
  - vector.reciprocal is only ~1e-5 accurate -> Newton-refined where it
    feeds quantization scales.
"""

import numpy as np
import ml_dtypes

# ---- problem geometry (hardcoded; harness contract) ----
B, S, D, O = 4, 2048, 2048, 8192
T = B * S                # 8192 token rows
NCORES = 8
OS = O // NCORES         # 1024 out features per core
TSH = T // NCORES        # 1024 token rows per core for phase-1 stats
P = 128
NT = T // P              # 64 token tiles
NSH = TSH // P           # 8 shard tiles
KD = D // P              # 16 contraction chunks
GN = 4                   # transpose groups of 4 blocks per psum tile
EPS = 1e-5
MAGIC = float(np.float32(1.5 * 2 ** 23))
CC_ROWS = TSH + 4        # stats rows + scalar rows (amax/wsum/wabs + pad)
CC_LEN = CC_ROWS * 2

_BUILT = None


def _build():
    from contextlib import ExitStack
    import concourse.bacc as bacc
    import concourse.tile as tile
    import concourse.bass as bass
    from concourse import mybir
    from concourse import bass_isa

    f32, bf16 = mybir.dt.float32, mybir.dt.bfloat16
    A = mybir.AluOpType
    AF = mybir.ActivationFunctionType

    nc = bacc.Bacc(None, target_bir_lowering=False, num_devices=NCORES)

    x_ext = nc.declare_dram_parameter("x", [T, D], f32, isOutput=False)
    xs_ext = nc.declare_dram_parameter("xs", [TSH, D], f32, isOutput=False)
    w_ext = nc.declare_dram_parameter("w", [OS, D], f32, isOutput=False)
    b_ext = nc.declare_dram_parameter("b", [OS], f32, isOutput=False)
    out_ext = nc.declare_dram_parameter("out", [T, OS], f32, isOutput=True)

    ident_dram = nc.inline_tensor(np.eye(P, dtype=ml_dtypes.bfloat16), name="identity_mat")
    cc_in = nc.dram_tensor("cc_in", [CC_LEN], f32)
    cc_out = nc.dram_tensor("cc_out", [NCORES, CC_LEN], f32, addr_space="Shared")

    def ap(t, offset, dims):
        return bass.AP(tensor=t.tensor if hasattr(t, "tensor") else t,
                       offset=offset, ap=[list(d) for d in dims])

    with ExitStack() as ctx:
        tc = ctx.enter_context(tile.TileContext(nc))
        sing = ctx.enter_context(tc.tile_pool(name="sing", bufs=1))
        small = ctx.enter_context(tc.tile_pool(name="small", bufs=4))
        xpool = ctx.enter_context(tc.tile_pool(name="xpool", bufs=3))
        wpool = ctx.enter_context(tc.tile_pool(name="wpool", bufs=2))
        wqpool = ctx.enter_context(tc.tile_pool(name="wqpool", bufs=2))
        vpool = ctx.enter_context(tc.tile_pool(name="vpool", bufs=3))
        rpool = ctx.enter_context(tc.tile_pool(name="rpool", bufs=2))
        mpool = ctx.enter_context(tc.tile_pool(name="mpool", bufs=2))
        qpool = ctx.enter_context(tc.tile_pool(name="qpool", bufs=3))
        qtpool = ctx.enter_context(tc.tile_pool(name="qtpool", bufs=3))
        opool = ctx.enter_context(tc.tile_pool(name="opool", bufs=2))
        topool = ctx.enter_context(tc.tile_pool(name="topool", bufs=3))
        pst = ctx.enter_context(tc.tile_pool(name="pst", bufs=2, space="PSUM"))
        psm = ctx.enter_context(tc.tile_pool(name="psm", bufs=6, space="PSUM"))

        ident = sing.tile([P, P], bf16)
        nc.sync.dma_start(out=ident, in_=ident_dram[:])

        cc_v = cc_in[:].rearrange("(r c) -> r c", c=2)

        # ---------- W shard partial sums (pre-gather) ----------
        wsums = sing.tile([P, NSH], f32)
        wabss = sing.tile([P, NSH], f32)
        for j in range(NSH):
            wt = wpool.tile([P, D], f32)
            nc.sync.dma_start(out=wt, in_=w_ext[P * j:P * (j + 1), :])
            nc.vector.tensor_reduce(out=wsums[:, j:j + 1], in_=wt,
                                    axis=mybir.AxisListType.X, op=A.add)
            nc.vector.tensor_reduce(out=wabss[:, j:j + 1], in_=wt,
                                    axis=mybir.AxisListType.X, op=A.add,
                                    apply_absolute_value=True)
        wsum1 = small.tile([P, 1], f32)
        nc.vector.tensor_reduce(out=wsum1, in_=wsums, axis=mybir.AxisListType.X, op=A.add)
        wabs1 = small.tile([P, 1], f32)
        nc.vector.tensor_reduce(out=wabs1, in_=wabss, axis=mybir.AxisListType.X, op=A.add)
        wsum_all = small.tile([P, 1], f32)
        nc.gpsimd.partition_all_reduce(wsum_all, wsum1, channels=P,
                                       reduce_op=bass_isa.ReduceOp.add)
        wabs_all = small.tile([P, 1], f32)
        nc.gpsimd.partition_all_reduce(wabs_all, wabs1, channels=P,
                                       reduce_op=bass_isa.ReduceOp.add)
        nc.sync.dma_start(out=cc_v[TSH:TSH + 1, 1:2], in_=wsum_all[0:1, 0:1])
        nc.sync.dma_start(out=cc_v[TSH + 1:TSH + 2, 0:1], in_=wabs_all[0:1, 0:1])

        # ---------- phase 1: row-sharded stats over xs ----------
        amax8 = sing.tile([P, NSH], f32)
        mv8 = sing.tile([P, NSH, 2], f32)
        xmax8 = sing.tile([P, NSH], f32)
        xmin8 = sing.tile([P, NSH], f32)
        for j in range(NSH):
            xt = xpool.tile([P, D], f32, tag="xtile")
            nc.sync.dma_start(out=xt, in_=xs_ext[P * j:P * (j + 1), :])
            stats = small.tile([P, 4, 6], f32)
            for g in range(4):
                nc.vector.bn_stats(out=stats[:, g, :], in_=xt[:, 512 * g:512 * (g + 1)])
            nc.vector.bn_aggr(out=mv8[:, j, :], in_=stats)
            nc.vector.tensor_reduce(out=xmax8[:, j:j + 1], in_=xt,
                                    axis=mybir.AxisListType.X, op=A.max)
            nc.vector.tensor_reduce(out=xmin8[:, j:j + 1], in_=xt,
                                    axis=mybir.AxisListType.X, op=A.min)
        mu8 = mv8[:, :, 0:1].rearrange("p n c -> p (n c)")
        var8 = mv8[:, :, 1:2].rearrange("p n c -> p (n c)")
        # rstd = 1/sqrt(var+eps), Newton-refined against var+eps (batched [P, NSH])
        ve8 = small.tile([P, NSH], f32)
        nc.vector.tensor_scalar(out=ve8, in0=var8, scalar1=EPS, scalar2=None, op0=A.add)
        sd8 = small.tile([P, NSH], f32)
        nc.scalar.activation(out=sd8, in_=ve8, func=AF.Sqrt)
        r08 = small.tile([P, NSH], f32)
        nc.vector.reciprocal(out=r08, in_=sd8)
        y28 = small.tile([P, NSH], f32)
        nc.vector.tensor_tensor(out=y28, in0=r08, in1=r08, op=A.mult)
        h8 = small.tile([P, NSH], f32)
        nc.vector.tensor_tensor(out=h8, in0=ve8, in1=y28, op=A.mult)
        nc.vector.tensor_scalar(out=h8, in0=h8, scalar1=-0.5, scalar2=1.5,
                                op0=A.mult, op1=A.add)
        stout = sing.tile([P, NSH, 2], f32)
        rsig8 = stout[:, :, 1:2].rearrange("p n c -> p (n c)")
        nc.vector.tensor_tensor(out=rsig8, in0=r08, in1=h8, op=A.mult)
        nc.vector.tensor_copy(out=stout[:, :, 0:1].rearrange("p n c -> p (n c)"), in_=mu8)
        # row absmax of normalized x (batched)
        a18 = small.tile([P, NSH], f32)
        nc.vector.tensor_tensor(out=a18, in0=xmax8, in1=mu8, op=A.subtract)
        a28 = small.tile([P, NSH], f32)
        nc.vector.tensor_tensor(out=a28, in0=mu8, in1=xmin8, op=A.subtract)
        nc.vector.tensor_tensor(out=a18, in0=a18, in1=a28, op=A.max)
        nc.vector.tensor_tensor(out=amax8, in0=a18, in1=rsig8, op=A.mult)
        # ship row stats in one DMA: cc row = 128j + p, col c -> offset 256j + 2p + c
        nc.sync.dma_start(out=ap(cc_in, 0, [[2, P], [2 * P, NSH], [1, 2]]), in_=stout)
        amax1 = small.tile([P, 1], f32)
        nc.vector.tensor_reduce(out=amax1, in_=amax8, axis=mybir.AxisListType.X, op=A.max)
        amax_all = small.tile([P, 1], f32)
        nc.gpsimd.partition_all_reduce(amax_all, amax1, channels=P,
                                       reduce_op=bass_isa.ReduceOp.max)
        nc.sync.dma_start(out=cc_v[TSH:TSH + 1, 0:1], in_=amax_all[0:1, 0:1])

        # ---------- the one collective ----------
        nc.gpsimd.collective_compute(
            "AllGather", A.bypass, replica_groups=[list(range(NCORES))],
            ins=[cc_in[:]], outs=[cc_out[:]],
        )

        # ---------- post-gather: global scalars + per-row scale/bias ----------
        stats_sb = sing.tile([P, NCORES, NSH, 2], f32)
        for rr in range(NCORES):
            nc.gpsimd.dma_start(
                out=stats_sb[:, rr, :, :],
                in_=ap(cc_out, rr * CC_LEN, [[2, P], [2 * P, NSH], [1, 2]]),
            )
        scal_b = sing.tile([P, 3, NCORES], f32)
        for rr in range(NCORES):
            nc.gpsimd.dma_start(
                out=scal_b[:, :, rr:rr + 1],
                in_=ap(cc_out, rr * CC_LEN + 2 * TSH, [[0, P], [1, 3]]),
            )
        gmax_b = sing.tile([P, 1], f32)
        nc.vector.tensor_reduce(out=gmax_b, in_=scal_b[:, 0:1, :],
                                axis=mybir.AxisListType.X, op=A.max)
        # scal_b[:, f, r] = cc_out[r, 2*TSH + f]:
        #   f=0 -> cc_v[TSH,0]=amax ; f=1 -> cc_v[TSH,1]=wsum ; f=2 -> cc_v[TSH+1,0]=wabs
        wsum_b = sing.tile([P, 1], f32)
        nc.vector.tensor_reduce(out=wsum_b, in_=scal_b[:, 1:2, :],
                                axis=mybir.AxisListType.X, op=A.add)
        wabs_b = sing.tile([P, 1], f32)
        nc.vector.tensor_reduce(out=wabs_b, in_=scal_b[:, 2:3, :],
                                axis=mybir.AxisListType.X, op=A.add)

        # S = 128/max (Newton-refined reciprocal)
        rm = sing.tile([P, 1], f32)
        nc.vector.reciprocal(out=rm, in_=gmax_b)
        tmn = sing.tile([P, 1], f32)
        nc.vector.tensor_tensor(out=tmn, in0=gmax_b, in1=rm, op=A.mult)
        nc.vector.tensor_scalar(out=tmn, in0=tmn, scalar1=-1.0, scalar2=2.0,
                                op0=A.mult, op1=A.add)
        nc.vector.tensor_tensor(out=rm, in0=rm, in1=tmn, op=A.mult)
        S_pp = sing.tile([P, 1], f32)
        nc.vector.tensor_scalar(out=S_pp, in0=rm, scalar1=128.0, scalar2=None, op0=A.mult)
        dequant_pp = sing.tile([P, 1], f32)
        nc.vector.tensor_scalar(out=dequant_pp, in0=gmax_b, scalar1=1.0 / 128.0,
                                scalar2=None, op0=A.mult)
        beta_pp = sing.tile([P, 1], f32)
        nc.vector.tensor_scalar(out=beta_pp, in0=wabs_b, scalar1=1.0 / float(D * O),
                                scalar2=None, op0=A.mult)
        alpha_pp = sing.tile([P, 1], f32)
        nc.vector.tensor_tensor(out=alpha_pp, in0=dequant_pp, in1=beta_pp, op=A.mult)
        negwm_pp = sing.tile([P, 1], f32)
        nc.vector.tensor_scalar(out=negwm_pp, in0=wsum_b, scalar1=-1.0 / float(D * O),
                                scalar2=None, op0=A.mult)

        # per-row scale s = rstd*S and bias bp = -mu*s  (full fp32 precision)
        mu_all = stats_sb[:].rearrange("p a b c -> p (a b) c")[:, :, 0:1]\
            .rearrange("p n c -> p (n c)")
        rs_all = stats_sb[:].rearrange("p a b c -> p (a b) c")[:, :, 1:2]\
            .rearrange("p n c -> p (n c)")
        s_all = sing.tile([P, NT], f32)
        nc.vector.tensor_scalar(out=s_all, in0=rs_all, scalar1=S_pp[:, 0:1],
                                scalar2=None, op0=A.mult)
        bp_all = sing.tile([P, NT], f32)
        nc.vector.tensor_tensor(out=bp_all, in0=mu_all, in1=s_all, op=A.mult)
        nc.vector.tensor_scalar(out=bp_all, in0=bp_all, scalar1=-1.0, scalar2=None,
                                op0=A.mult)

        # ---------- W shard: sign-binarize + transpose ----------
        wqT = sing.tile([P, KD, OS], bf16)
        for j in range(NSH):
            wt = wpool.tile([P, D], f32)
            nc.sync.dma_start(out=wt, in_=w_ext[P * j:P * (j + 1), :])
            wq = wqpool.tile([P, D], bf16)
            nc.scalar.activation(out=wq, in_=wt, func=AF.Sign, bias=negwm_pp, scale=1.0)
            for mg in range(KD // GN):
                pt = pst.tile([P, GN * P], bf16, tag="ptrans")
                for g in range(GN):
                    k = GN * mg + g
                    nc.tensor.transpose(pt[:, P * g:P * (g + 1)],
                                        wq[:, P * k:P * (k + 1)], ident)
                nc.scalar.copy(
                    out=wqT[:, GN * mg:GN * (mg + 1), P * j:P * (j + 1)],
                    in_=pt[:].rearrange("p (g t) -> p g t", g=GN))

        # bias broadcast [P, OS]
        bias_bc = sing.tile([P, OS], f32)
        nc.gpsimd.dma_start(out=bias_bc, in_=ap(b_ext, 0, [[0, P], [1, OS]]))

        # ---------- phase 2: quantize + matmul ----------
        for j in range(NT):
            xt = xpool.tile([P, D], f32, tag="xtile")
            nc.sync.dma_start(out=xt, in_=x_ext[P * j:P * (j + 1), :])
            v1 = vpool.tile([P, D], f32)
            nc.scalar.activation(out=v1, in_=xt, func=AF.Identity,
                                 bias=bp_all[:, j:j + 1], scale=s_all[:, j:j + 1])
            r = rpool.tile([P, D], f32)
            nc.vector.tensor_scalar(out=r, in0=v1, scalar1=MAGIC, scalar2=MAGIC,
                                    op0=A.add, op1=A.subtract)
            m256 = mpool.tile([P, D], f32)
            nc.vector.tensor_scalar(out=m256, in0=v1, scalar1=127.5, scalar2=256.0,
                                    op0=A.is_ge, op1=A.mult)
            q = qpool.tile([P, D], bf16)
            nc.vector.tensor_sub(out=q, in0=r, in1=m256)
            qT = qtpool.tile([P, KD, P], bf16)
            for mg in range(KD // GN):
                pt = pst.tile([P, GN * P], bf16, tag="ptrans")
                for g in range(GN):
                    k = GN * mg + g
                    nc.tensor.transpose(pt[:, P * g:P * (g + 1)],
                                        q[:, P * k:P * (k + 1)], ident)
                nc.scalar.copy(out=qT[:, GN * mg:GN * (mg + 1), :],
                               in_=pt[:].rearrange("p (g t) -> p g t", g=GN))
            out_sb = opool.tile([P, OS], f32)
            pm0 = psm.tile([P, 512], f32, tag="pmat")
            pm1 = psm.tile([P, 512], f32, tag="pmat")
            for k in range(KD):
                nc.tensor.matmul(pm0, lhsT=qT[:, k, :], rhs=wqT[:, k, 0:512],
                                 start=(k == 0), stop=(k == KD - 1))
                nc.tensor.matmul(pm1, lhsT=qT[:, k, :], rhs=wqT[:, k, 512:1024],
                                 start=(k == 0), stop=(k == KD - 1))
            to0 = topool.tile([P, 512], f32)
            nc.scalar.activation(out=to0, in_=pm0, func=AF.Copy, scale=alpha_pp[:, 0:1])
            nc.vector.tensor_add(out=out_sb[:, 0:512], in0=to0, in1=bias_bc[:, 0:512])
            to1 = topool.tile([P, 512], f32)
            nc.scalar.activation(out=to1, in_=pm1, func=AF.Copy, scale=alpha_pp[:, 0:1])
            nc.vector.tensor_add(out=out_sb[:, 512:1024], in0=to1, in1=bias_bc[:, 512:1024])
            nc.sync.dma_start(out=out_ext[P * j:P * (j + 1), :], in_=out_sb)

    nc.finalize()
    return nc


def kernel(x, weights, bias):
    global _BUILT
    from concourse.bass_utils import run_bass_kernel_spmd

    if _BUILT is None:
        _BUILT = _build()
    nc = _BUILT

    xf = np.ascontiguousarray(x.reshape(T, D), dtype=np.float32)
    in_maps = []
    for i in range(NCORES):
        in_maps.append({
            "x": xf,
            "xs": np.ascontiguousarray(xf[TSH * i:TSH * (i + 1)]),
            "w": np.ascontiguousarray(weights[OS * i:OS * (i + 1)], dtype=np.float32),
            "b": np.ascontiguousarray(bias[OS * i:OS * (i + 1)], dtype=np.float32),
        })
    res = run_bass_kernel_spmd(nc, in_maps, core_ids=list(range(NCORES)))
    out = np.concatenate([res.results[i]["out"] for i in range(NCORES)], axis=1)
    return out.reshape(B, S, O).astype(np.float32)


if __name__ == "__main__":
    rng = np.random.RandomState(0)
    x = rng.randn(B, S, D).astype(np.float32)
    w = rng.randn(O, D).astype(np.float32)
    b = rng.randn(O).astype(np.float32)
    out = kernel(x, w, b)
    print("out", out.shape, out.dtype, float(np.abs(out).mean()))


# revision 8
# speedup vs baseline: 1.1608x; 1.1608x over previous
"""BitLinear forward on 8 Trainium2 NeuronCores (self-contained).

Reference computation (see problem):
    input_norm = layernorm(x)                     # per-token, eps=1e-5, no affine
    max_val    = absmax(input_norm)               # GLOBAL over all of x
    q          = int8-wrap(round(input_norm * 128/max_val))
    wq         = sign(W - mean(W))                # global mean
    out        = (q @ wq.T) * (max_val/128 * mean|W|) + bias

Distribution (8 cores, SPMD single NEFF):
  - Tensor-parallel: W/bias sharded along out_features (1024 rows/core);
    every core computes q for all 8192 tokens and its 1024 output columns.
  - Phase-1 stats (per-row mean/rstd + row absmax, W partial sums) are
    row-sharded (1/8 of tokens per core) and exchanged with ONE AllGather
    of 8.2 KB per core; global absmax / sum(W) / sum|W| ride along as
    extra rows and are reduced locally after the gather.
  - Matmul runs in bf16 (q in [-128,127] and wq in {-1,1} are exact in
    bf16; PE accumulates fp32 -> bit-exact integer arithmetic).

Numerics notes:
  - round() is implemented with the fp32 magic-constant trick
    (v + 1.5*2^23 - 1.5*2^23 == RNE round for |v| <= 2^22).
  - the int8 two's-complement wrap (128 -> -128) is q = round(v) - 256*(v>=127.5).
  - vector.reciprocal is only ~1e-5 accurate -> Newton-refined where it
    feeds quantization scales.
"""

import numpy as np
import ml_dtypes

# ---- problem geometry (hardcoded; harness contract) ----
B, S, D, O = 4, 2048, 2048, 8192
T = B * S                # 8192 token rows
NCORES = 8
OS = O // NCORES         # 1024 out features per core
TSH = T // NCORES        # 1024 token rows per core for phase-1 stats
P = 128
NT = T // P              # 64 token tiles
NSH = TSH // P           # 8 shard tiles
KD = D // P              # 16 contraction chunks
GN = 4                   # transpose groups of 4 blocks per psum tile
EPS = 1e-5
MAGIC = float(np.float32(1.5 * 2 ** 23))
CC_ROWS = TSH + 4        # stats rows + scalar rows (amax/wsum/wabs + pad)
CC_LEN = CC_ROWS * 2

_BUILT = None


def _build():
    from contextlib import ExitStack
    import concourse.bacc as bacc
    import concourse.tile as tile
    import concourse.bass as bass
    from concourse import mybir
    from concourse import bass_isa

    f32, bf16 = mybir.dt.float32, mybir.dt.bfloat16
    A = mybir.AluOpType
    AF = mybir.ActivationFunctionType

    nc = bacc.Bacc(None, target_bir_lowering=False, num_devices=NCORES)

    x_ext = nc.declare_dram_parameter("x", [T, D], f32, isOutput=False)
    xs_ext = nc.declare_dram_parameter("xs", [TSH, D], f32, isOutput=False)
    w_ext = nc.declare_dram_parameter("w", [OS, D], f32, isOutput=False)
    b_ext = nc.declare_dram_parameter("b", [OS], f32, isOutput=False)
    out_ext = nc.declare_dram_parameter("out", [T, OS], f32, isOutput=True)

    ident_dram = nc.inline_tensor(np.eye(P, dtype=ml_dtypes.bfloat16), name="identity_mat")
    cc_in = nc.dram_tensor("cc_in", [CC_LEN], f32)
    cc_out = nc.dram_tensor("cc_out", [NCORES, CC_LEN], f32, addr_space="Shared")

    def ap(t, offset, dims):
        return bass.AP(tensor=t.tensor if hasattr(t, "tensor") else t,
                       offset=offset, ap=[list(d) for d in dims])

    with ExitStack() as ctx:
        tc = ctx.enter_context(tile.TileContext(nc))
        sing = ctx.enter_context(tc.tile_pool(name="sing", bufs=1))
        small = ctx.enter_context(tc.tile_pool(name="small", bufs=4))
        xpool = ctx.enter_context(tc.tile_pool(name="xpool", bufs=3))
        wpool = ctx.enter_context(tc.tile_pool(name="wpool", bufs=2))
        wqpool = ctx.enter_context(tc.tile_pool(name="wqpool", bufs=2))
        vpool = ctx.enter_context(tc.tile_pool(name="vpool", bufs=3))
        rpool = ctx.enter_context(tc.tile_pool(name="rpool", bufs=2))
        mpool = ctx.enter_context(tc.tile_pool(name="mpool", bufs=2))
        qpool = ctx.enter_context(tc.tile_pool(name="qpool", bufs=3))
        qtpool = ctx.enter_context(tc.tile_pool(name="qtpool", bufs=3))
        opool = ctx.enter_context(tc.tile_pool(name="opool", bufs=2))
        topool = ctx.enter_context(tc.tile_pool(name="topool", bufs=3))
        pst = ctx.enter_context(tc.tile_pool(name="pst", bufs=4, space="PSUM"))
        psm = ctx.enter_context(tc.tile_pool(name="psm", bufs=4, space="PSUM"))

        ident = sing.tile([P, P], bf16)
        nc.sync.dma_start(out=ident, in_=ident_dram[:])

        cc_v = cc_in[:].rearrange("(r c) -> r c", c=2)

        # ---------- W shard partial sums (pre-gather) ----------
        wsums = sing.tile([P, NSH], f32)
        wabss = sing.tile([P, NSH], f32)
        for j in range(NSH):
            wt = wpool.tile([P, D], f32)
            nc.sync.dma_start(out=wt, in_=w_ext[P * j:P * (j + 1), :])
            nc.vector.tensor_reduce(out=wsums[:, j:j + 1], in_=wt,
                                    axis=mybir.AxisListType.X, op=A.add)
            nc.vector.tensor_reduce(out=wabss[:, j:j + 1], in_=wt,
                                    axis=mybir.AxisListType.X, op=A.add,
                                    apply_absolute_value=True)
        wsum1 = small.tile([P, 1], f32)
        nc.vector.tensor_reduce(out=wsum1, in_=wsums, axis=mybir.AxisListType.X, op=A.add)
        wabs1 = small.tile([P, 1], f32)
        nc.vector.tensor_reduce(out=wabs1, in_=wabss, axis=mybir.AxisListType.X, op=A.add)
        wsum_all = small.tile([P, 1], f32)
        nc.gpsimd.partition_all_reduce(wsum_all, wsum1, channels=P,
                                       reduce_op=bass_isa.ReduceOp.add)
        wabs_all = small.tile([P, 1], f32)
        nc.gpsimd.partition_all_reduce(wabs_all, wabs1, channels=P,
                                       reduce_op=bass_isa.ReduceOp.add)
        nc.sync.dma_start(out=cc_v[TSH:TSH + 1, 1:2], in_=wsum_all[0:1, 0:1])
        nc.sync.dma_start(out=cc_v[TSH + 1:TSH + 2, 0:1], in_=wabs_all[0:1, 0:1])

        # ---------- phase 1: row-sharded stats over xs ----------
        amax8 = sing.tile([P, NSH], f32)
        mv8 = sing.tile([P, NSH, 2], f32)
        xmax8 = sing.tile([P, NSH], f32)
        xmin8 = sing.tile([P, NSH], f32)
        for j in range(NSH):
            xt = xpool.tile([P, D], f32, tag="xtile")
            nc.sync.dma_start(out=xt, in_=xs_ext[P * j:P * (j + 1), :])
            stats = small.tile([P, 4, 6], f32)
            for g in range(4):
                nc.vector.bn_stats(out=stats[:, g, :], in_=xt[:, 512 * g:512 * (g + 1)])
            nc.vector.bn_aggr(out=mv8[:, j, :], in_=stats)
            nc.vector.tensor_reduce(out=xmax8[:, j:j + 1], in_=xt,
                                    axis=mybir.AxisListType.X, op=A.max)
            nc.vector.tensor_reduce(out=xmin8[:, j:j + 1], in_=xt,
                                    axis=mybir.AxisListType.X, op=A.min)
        mu8 = mv8[:, :, 0:1].rearrange("p n c -> p (n c)")
        var8 = mv8[:, :, 1:2].rearrange("p n c -> p (n c)")
        # rstd = 1/sqrt(var+eps), Newton-refined against var+eps (batched [P, NSH])
        ve8 = small.tile([P, NSH], f32)
        nc.vector.tensor_scalar(out=ve8, in0=var8, scalar1=EPS, scalar2=None, op0=A.add)
        sd8 = small.tile([P, NSH], f32)
        nc.scalar.activation(out=sd8, in_=ve8, func=AF.Sqrt)
        r08 = small.tile([P, NSH], f32)
        nc.vector.reciprocal(out=r08, in_=sd8)
        y28 = small.tile([P, NSH], f32)
        nc.vector.tensor_tensor(out=y28, in0=r08, in1=r08, op=A.mult)
        h8 = small.tile([P, NSH], f32)
        nc.vector.tensor_tensor(out=h8, in0=ve8, in1=y28, op=A.mult)
        nc.vector.tensor_scalar(out=h8, in0=h8, scalar1=-0.5, scalar2=1.5,
                                op0=A.mult, op1=A.add)
        stout = sing.tile([P, NSH, 2], f32)
        rsig8 = stout[:, :, 1:2].rearrange("p n c -> p (n c)")
        nc.vector.tensor_tensor(out=rsig8, in0=r08, in1=h8, op=A.mult)
        nc.vector.tensor_copy(out=stout[:, :, 0:1].rearrange("p n c -> p (n c)"), in_=mu8)
        # row absmax of normalized x (batched)
        a18 = small.tile([P, NSH], f32)
        nc.vector.tensor_tensor(out=a18, in0=xmax8, in1=mu8, op=A.subtract)
        a28 = small.tile([P, NSH], f32)
        nc.vector.tensor_tensor(out=a28, in0=mu8, in1=xmin8, op=A.subtract)
        nc.vector.tensor_tensor(out=a18, in0=a18, in1=a28, op=A.max)
        nc.vector.tensor_tensor(out=amax8, in0=a18, in1=rsig8, op=A.mult)
        # ship row stats in one DMA: cc row = 128j + p, col c -> offset 256j + 2p + c
        nc.sync.dma_start(out=ap(cc_in, 0, [[2, P], [2 * P, NSH], [1, 2]]), in_=stout)
        amax1 = small.tile([P, 1], f32)
        nc.vector.tensor_reduce(out=amax1, in_=amax8, axis=mybir.AxisListType.X, op=A.max)
        amax_all = small.tile([P, 1], f32)
        nc.gpsimd.partition_all_reduce(amax_all, amax1, channels=P,
                                       reduce_op=bass_isa.ReduceOp.max)
        nc.sync.dma_start(out=cc_v[TSH:TSH + 1, 0:1], in_=amax_all[0:1, 0:1])

        # ---------- the one collective ----------
        nc.gpsimd.collective_compute(
            "AllGather", A.bypass, replica_groups=[list(range(NCORES))],
            ins=[cc_in[:]], outs=[cc_out[:]],
        )

        # ---------- post-gather: global scalars + per-row scale/bias ----------
        stats_sb = sing.tile([P, NCORES, NSH, 2], f32)
        for rr in range(NCORES):
            nc.gpsimd.dma_start(
                out=stats_sb[:, rr, :, :],
                in_=ap(cc_out, rr * CC_LEN, [[2, P], [2 * P, NSH], [1, 2]]),
            )
        scal_b = sing.tile([P, 3, NCORES], f32)
        for rr in range(NCORES):
            nc.gpsimd.dma_start(
                out=scal_b[:, :, rr:rr + 1],
                in_=ap(cc_out, rr * CC_LEN + 2 * TSH, [[0, P], [1, 3]]),
            )
        gmax_b = sing.tile([P, 1], f32)
        nc.vector.tensor_reduce(out=gmax_b, in_=scal_b[:, 0:1, :],
                                axis=mybir.AxisListType.X, op=A.max)
        # scal_b[:, f, r] = cc_out[r, 2*TSH + f]:
        #   f=0 -> cc_v[TSH,0]=amax ; f=1 -> cc_v[TSH,1]=wsum ; f=2 -> cc_v[TSH+1,0]=wabs
        wsum_b = sing.tile([P, 1], f32)
        nc.vector.tensor_reduce(out=wsum_b, in_=scal_b[:, 1:2, :],
                                axis=mybir.AxisListType.X, op=A.add)
        wabs_b = sing.tile([P, 1], f32)
        nc.vector.tensor_reduce(out=wabs_b, in_=scal_b[:, 2:3, :],
                                axis=mybir.AxisListType.X, op=A.add)

        # S = 128/max (Newton-refined reciprocal)
        rm = sing.tile([P, 1], f32)
        nc.vector.reciprocal(out=rm, in_=gmax_b)
        tmn = sing.tile([P, 1], f32)
        nc.vector.tensor_tensor(out=tmn, in0=gmax_b, in1=rm, op=A.mult)
        nc.vector.tensor_scalar(out=tmn, in0=tmn, scalar1=-1.0, scalar2=2.0,
                                op0=A.mult, op1=A.add)
        nc.vector.tensor_tensor(out=rm, in0=rm, in1=tmn, op=A.mult)
        S_pp = sing.tile([P, 1], f32)
        nc.vector.tensor_scalar(out=S_pp, in0=rm, scalar1=128.0, scalar2=None, op0=A.mult)
        dequant_pp = sing.tile([P, 1], f32)
        nc.vector.tensor_scalar(out=dequant_pp, in0=gmax_b, scalar1=1.0 / 128.0,
                                scalar2=None, op0=A.mult)
        beta_pp = sing.tile([P, 1], f32)
        nc.vector.tensor_scalar(out=beta_pp, in0=wabs_b, scalar1=1.0 / float(D * O),
                                scalar2=None, op0=A.mult)
        alpha_pp = sing.tile([P, 1], f32)
        nc.vector.tensor_tensor(out=alpha_pp, in0=dequant_pp, in1=beta_pp, op=A.mult)
        negwm_pp = sing.tile([P, 1], f32)
        nc.vector.tensor_scalar(out=negwm_pp, in0=wsum_b, scalar1=-1.0 / float(D * O),
                                scalar2=None, op0=A.mult)

        # per-row scale s = rstd*S and bias bp = -mu*s  (full fp32 precision)
        mu_all = stats_sb[:].rearrange("p a b c -> p (a b) c")[:, :, 0:1]\
            .rearrange("p n c -> p (n c)")
        rs_all = stats_sb[:].rearrange("p a b c -> p (a b) c")[:, :, 1:2]\
            .rearrange("p n c -> p (n c)")
        s_all = sing.tile([P, NT], f32)
        nc.vector.tensor_scalar(out=s_all, in0=rs_all, scalar1=S_pp[:, 0:1],
                                scalar2=None, op0=A.mult)
        bp_all = sing.tile([P, NT], f32)
        nc.vector.tensor_tensor(out=bp_all, in0=mu_all, in1=s_all, op=A.mult)
        nc.vector.tensor_scalar(out=bp_all, in0=bp_all, scalar1=-1.0, scalar2=None,
                                op0=A.mult)

        # ---------- W shard: sign-binarize + transpose ----------
        wqT = sing.tile([P, KD, OS], bf16)
        for j in range(NSH):
            wt = wpool.tile([P, D], f32)
            nc.sync.dma_start(out=wt, in_=w_ext[P * j:P * (j + 1), :])
            wq = wqpool.tile([P, D], bf16)
            nc.scalar.activation(out=wq, in_=wt, func=AF.Sign, bias=negwm_pp, scale=1.0)
            for mg in range(KD // GN):
                pt = pst.tile([P, GN * P], bf16, tag="ptrans")
                for g in range(GN):
                    k = GN * mg + g
                    nc.tensor.transpose(pt[:, P * g:P * (g + 1)],
                                        wq[:, P * k:P * (k + 1)], ident)
                nc.scalar.copy(
                    out=wqT[:, GN * mg:GN * (mg + 1), P * j:P * (j + 1)],
                    in_=pt[:].rearrange("p (g t) -> p g t", g=GN))

        # bias broadcast [P, OS]
        bias_bc = sing.tile([P, OS], f32)
        nc.gpsimd.dma_start(out=bias_bc, in_=ap(b_ext, 0, [[0, P], [1, OS]]))

        # ---------- phase 2: quantize + matmul ----------
        for j in range(NT):
            xt = xpool.tile([P, D], f32, tag="xtile")
            nc.sync.dma_start(out=xt, in_=x_ext[P * j:P * (j + 1), :])
            v1 = vpool.tile([P, D], f32)
            nc.vector.tensor_scalar(out=v1, in0=xt, scalar1=s_all[:, j:j + 1],
                                    scalar2=bp_all[:, j:j + 1], op0=A.mult, op1=A.add)
            r = rpool.tile([P, D], f32)
            nc.vector.tensor_scalar(out=r, in0=v1, scalar1=MAGIC, scalar2=MAGIC,
                                    op0=A.add, op1=A.subtract)
            m256 = mpool.tile([P, D], f32)
            nc.vector.tensor_scalar(out=m256, in0=v1, scalar1=127.5, scalar2=256.0,
                                    op0=A.is_ge, op1=A.mult)
            q = qpool.tile([P, D], bf16)
            nc.vector.tensor_sub(out=q, in0=r, in1=m256)
            qT = qtpool.tile([P, KD, P], bf16)
            for mg in range(KD // GN):
                pt = pst.tile([P, GN * P], bf16, tag="ptrans")
                for g in range(GN):
                    k = GN * mg + g
                    nc.tensor.transpose(pt[:, P * g:P * (g + 1)],
                                        q[:, P * k:P * (k + 1)], ident)
                nc.scalar.copy(out=qT[:, GN * mg:GN * (mg + 1), :],
                               in_=pt[:].rearrange("p (g t) -> p g t", g=GN))
            out_sb = opool.tile([P, OS], f32)
            pm0 = psm.tile([P, 512], f32, tag="pmat")
            pm1 = psm.tile([P, 512], f32, tag="pmat")
            for k in range(KD):
                nc.tensor.matmul(pm0, lhsT=qT[:, k, :], rhs=wqT[:, k, 0:512],
                                 start=(k == 0), stop=(k == KD - 1))
                nc.tensor.matmul(pm1, lhsT=qT[:, k, :], rhs=wqT[:, k, 512:1024],
                                 start=(k == 0), stop=(k == KD - 1))
            to0 = topool.tile([P, 512], f32)
            nc.scalar.activation(out=to0, in_=pm0, func=AF.Copy, scale=alpha_pp[:, 0:1])
            nc.vector.tensor_add(out=out_sb[:, 0:512], in0=to0, in1=bias_bc[:, 0:512])
            to1 = topool.tile([P, 512], f32)
            nc.scalar.activation(out=to1, in_=pm1, func=AF.Copy, scale=alpha_pp[:, 0:1])
            nc.vector.tensor_add(out=out_sb[:, 512:1024], in0=to1, in1=bias_bc[:, 512:1024])
            nc.sync.dma_start(out=out_ext[P * j:P * (j + 1), :], in_=out_sb)

    nc.finalize()
    return nc


def kernel(x, weights, bias):
    global _BUILT
    from concourse.bass_utils import run_bass_kernel_spmd

    if _BUILT is None:
        _BUILT = _build()
    nc = _BUILT

    xf = np.ascontiguousarray(x.reshape(T, D), dtype=np.float32)
    in_maps = []
    for i in range(NCORES):
        in_maps.append({
            "x": xf,
            "xs": np.ascontiguousarray(xf[TSH * i:TSH * (i + 1)]),
            "w": np.ascontiguousarray(weights[OS * i:OS * (i + 1)], dtype=np.float32),
            "b": np.ascontiguousarray(bias[OS * i:OS * (i + 1)], dtype=np.float32),
        })
    res = run_bass_kernel_spmd(nc, in_maps, core_ids=list(range(NCORES)))
    out = np.concatenate([res.results[i]["out"] for i in range(NCORES)], axis=1)
    return out.reshape(B, S, O).astype(np.float32)


if __name__ == "__main__":
    rng = np.random.RandomState(0)
    x = rng.randn(B, S, D).astype(np.float32)
    w = rng.randn(O, D).astype(np.float32)
    b = rng.randn(O).astype(np.float32)
    out = kernel(x, w, b)
    print("out", out.shape, out.dtype, float(np.abs(out).mean()))
